# revision 34
# baseline (speedup 1.0000x reference)
"""Local2d (locally-connected conv, unshared weights) Trainium2 kernel.

Problem: out[b,o,h,w] = sum_{i,k,l} weight[o,h,w,i,k,l] * xpad[b,i,h+k,w+l] + bias[o,h,w]
  x: [64, 64, 32, 32] f32, weight: [128, 32, 32, 64, 3, 3] f32, bias: [128, 32, 32] f32
  out: [64, 128, 32, 32] f32

Strategy: shard the 32 output rows h across 8 cores (4 rows each). Each output
location (h,w) is an independent GEMM: [o=128] x [ikl=576] @ [ikl=576] x [b=64],
chunked as 3 K=128 matmuls (taps k in {0,1} paired with equal l on the partition
dim) plus 3 K=65 matmuls (k=2), PSUM-accumulated. The weight tensor is the whole
ballgame: 75.5M elements used exactly once each, so HBM traffic ~= weight bytes
and the kernel is DMA-bound. All weights ship as TRN float8e3 (e3m4: 4
mantissa bits) scaled into the normal range and clipped to +-15.5, halving
weight DMA vs fp16. The x side is mixed per tap group: the pm stream (k0/k1,
2/3 of the x bytes) also ships e3m4 scaled x2 with its weight chunks scaled
x32 so partial products land on the common x64 scale, while the p2 stream
(k2 + the exact ones/bias lane) stays fp16 with weights x64 (mixed-dtype
matmul is allowed and full-rate). Error budget: weight quantization ~1.33e-2,
pm-x quantization adds quadrature to 1.72e-2 measured, vs 1.89e-2 if ALL x
were fp8 (too thin against the 2e-2 gate) - e3m4 is mantissa-limited, scale
tuning doesn't help further. Bias rides as the 65th partition row of the k=2
chunk (weight side: bias*64 on l=2; x side: a fp16 ones-lane inside p2) -
adding it as a separate K=1 matmul costs ~290ns PER MATMUL of PE serial time
(+37us!), and per-w PSUM reads on the 0.96 GHz DVE cost 120 fixed cycles
each, so PSUM drains through wide [128, 8*64] DVE copies instead. Host ships
raw padded x-row windows (1 DMA each, partition dim always a single source
axis - merged-source partition DMAs crash the device). On-chip, DVE expands
each x-row window into patch-shaped tiles with 3 shifted copies, so matmuls
read non-overlapping slices (clean Tile dependency graph; reading overlapping
w+l windows directly from the row tile is 3x slower). Output is fp16 scaled
x64, divided back on host. ~13.8MB DMA per core; cost-model steady state is
exactly DMA-busy-bound (38.4us/rep at the model's 360GB/s, zero bubbles).
Measured ~27.8us on a quiet epoch vs the fp16 baseline's 62us (shared-tenant
HBM makes any single print load-dependent); rel err 1.7186e-2 vs the 2e-2
gate.
"""

import os
import numpy as np

B, C_IN, C_OUT, KS, H, W = 64, 64, 128, 3, 32, 32
H_OUT, W_OUT = 32, 32
N_CORES = 8
H_PER = H_OUT // N_CORES  # 4
WSCALE = 64.0  # combined product scale: every chunk's partials are 64*(w@x)
X_SCALE = 2.0  # pm x rows pre-scale into e3m4 normal range (max 5.2*2 < 15.5)
WM_SCALE = WSCALE / X_SCALE  # wm weights scale, so wm*pm partials land on x64
E3M4_MAX = 15.5

_NC_CACHE = {}
_RUNNER_CACHE = {}
_LAST_IN_MAPS = None
LAST_RESULT = None


def _split_multiwaits(nc):
    """This container's walrus accepts at most ONE sync-wait per instruction.
    Hoist extra waits onto single-wait NoOps on the same engine, inserted
    immediately before (engine streams are in-order, sem waits are >=-monotonic,
    so this is semantics-preserving)."""
    import concourse.mybir as mybir

    ctr = 0
    hist = {}
    for f in nc.m.functions:
        for blk in f.blocks:
            insts = list(blk.instructions)
            changed = False
            newlist = []
            for inst in insts:
                si = inst.sync_info
                if si is not None and si.on_wait and len(si.on_wait) > 1:
                    tname = type(inst).__name__
                    hist[tname] = hist.get(tname, 0) + 1
                    waits = list(si.on_wait)
                    for wt in waits[:-1]:
                        nop = mybir.InstNoOp(name=f"splitwait-{ctr}", ins=[], outs=[])
                        ctr += 1
                        nop.engine = inst.engine
                        nop.sync_info = mybir.SyncInfo(on_wait=[wt], on_update=[])
                        newlist.append(nop)
                    inst.sync_info = mybir.SyncInfo(
                        on_wait=[waits[-1]], on_update=list(si.on_update or [])
                    )
                    changed = True
                newlist.append(inst)
            if changed:
                blk.instructions = newlist
    if os.environ.get("K_DEBUG"):
        print(f"split_multiwaits: {ctr} extra waits hoisted; by type: {hist}")
    return ctr


def _build_nc(reps=1):
    import concourse.bass as bass
    import concourse.mybir as mybir
    import concourse.tile as tile

    variant = os.environ.get("K_VARIANT", "v3")
    dt_pm = mybir.dt.float8e3  # k0/k1 x rows: e3m4 (x2), wm compensates (x32)
    dt_p2 = mybir.dt.float16  # k2 x rows + exact ones/bias lane stay fp16
    dt_w = mybir.dt.float8e3
    nc = bass.Bass()
    # Tap-paired scheme: chunks pair taps with EQUAL l and k in {0,1} on the
    # partition dim (both halves then read the same free offset w+l), plus a
    # K=65 chunk for k=2 whose 65th row carries the bias (weights side: bias*64
    # on l=2, zeros on l=0,1; x side: a ones-row shipped inside p2). Host
    # pre-merges (k,i)->ki and pre-windows x rows so every DMA partition dim is
    # a single source axis.
    C2 = C_IN + 1  # k=2 chunk partition count: 64 weights + 1 bias row
    wm_d = nc.dram_tensor(
        "wm", [H_PER, KS, 2 * C_IN, W_OUT, C_OUT], dt_w, kind="ExternalInput"
    )
    w2_d = nc.dram_tensor(
        "w2", [H_PER, KS, C2, W_OUT, C_OUT], dt_w, kind="ExternalInput"
    )
    pm_d = nc.dram_tensor(
        "pm", [H_PER, 2 * C_IN, W + 2, B], dt_pm, kind="ExternalInput"
    )
    p2_d = nc.dram_tensor(
        "p2", [H_PER, C2, W + 2, B], dt_p2, kind="ExternalInput"
    )
    o_d = nc.dram_tensor(
        "out", [C_OUT, H_PER, W_OUT, B], mybir.dt.float16, kind="ExternalOutput"
    )

    WG = 8  # w's per PSUM bank (one [128, 8*64] f32 tile = 2KB/partition = 1 bank)

    wbufs = int(os.environ.get("K_WBUFS", "2"))
    psbufs = int(os.environ.get("K_PSBUFS", "4"))
    with tile.TileContext(nc) as tc:
        with (
            tc.tile_pool(name="wp", bufs=wbufs) as wp,
            tc.tile_pool(name="pp", bufs=wbufs) as pp,
            tc.tile_pool(name="op", bufs=2) as op,
            tc.tile_pool(name="psp", bufs=psbufs, space="PSUM") as psp,
        ):
            for rep in range(reps):
                for h in range(H_PER):
                    # alternate the two HWDGE rings between the big streams
                    weng = nc.sync if h % 2 == 0 else nc.scalar
                    peng = nc.scalar if h % 2 == 0 else nc.sync
                    wm = wp.tile(
                        [128, KS, W_OUT, C_OUT], dt_w, tag="wm", name=f"wm_{rep}_{h}"
                    )
                    weng.dma_start(
                        wm[:], wm_d[h].rearrange("l p w o -> p l w o")
                    )
                    w2 = wp.tile(
                        [C2, KS, W_OUT, C_OUT], dt_w, tag="w2", name=f"w2_{rep}_{h}"
                    )
                    peng.dma_start(w2[:], w2_d[h].rearrange("l p w o -> p l w o"))
                    t01 = pp.tile(
                        [128, W + 2, B], dt_pm, tag="t01", name=f"t01_{rep}_{h}"
                    )
                    peng.dma_start(t01[:], pm_d[h])
                    t2 = pp.tile(
                        [C2, W + 2, B], dt_p2, tag="t2", name=f"t2_{rep}_{h}"
                    )
                    peng.dma_start(t2[:], p2_d[h])
                    # expand x-row windows into patch-shaped tiles on-chip
                    # (within-partition shifted copies) so matmuls read
                    # non-overlapping slices while x rows travel over DMA only
                    # once per h.
                    pl01 = pp.tile(
                        [128, KS, W_OUT, B], dt_pm, tag="pl01",
                        name=f"pl01_{rep}_{h}",
                    )
                    pl2 = pp.tile(
                        [C2, KS, W_OUT, B], dt_p2, tag="pl2",
                        name=f"pl2_{rep}_{h}",
                    )
                    for l in range(KS):
                        nc.vector.tensor_copy(
                            pl01[:, l, :, :], t01[:, l : l + W_OUT, :]
                        )
                        nc.vector.tensor_copy(
                            pl2[:, l, :, :], t2[:, l : l + W_OUT, :]
                        )
                    ot = op.tile(
                        [C_OUT, W_OUT, B], mybir.dt.float16, tag="ot",
                        name=f"ot_{rep}_{h}",
                    )
                    for wg in range(W_OUT // WG):
                        ps = psp.tile(
                            [C_OUT, WG, B], mybir.dt.float32, tag="ps",
                            name=f"ps_{rep}_{h}_{wg}",
                        )
                        for wi in range(WG):
                            w = wg * WG + wi
                            if variant == "p1":  # timing probe: 1 matmul/loc
                                nc.tensor.matmul(
                                    ps[:, wi, :],
                                    wm[:, 0, w, :],
                                    pl01[:, 0, w, :],
                                    start=True,
                                    stop=True,
                                )
                                continue
                            for l in range(KS):
                                nc.tensor.matmul(
                                    ps[:, wi, :],
                                    wm[:, l, w, :],
                                    pl01[:, l, w, :],
                                    start=(l == 0),
                                    stop=False,
                                )
                            for l in range(KS):
                                nc.tensor.matmul(
                                    ps[:, wi, :],
                                    w2[:, l, w, :],
                                    pl2[:, l, w, :],
                                    start=False,
                                    stop=(l == KS - 1),
                                )
                        nc.vector.tensor_copy(
                            ot[:, wg * WG : (wg + 1) * WG, :], ps[:]
                        )
                    nc.gpsimd.dma_start(o_d[:, h], ot[:])

    _split_multiwaits(nc)
    return nc


def _get_nc(reps=1):
    if reps not in _NC_CACHE:
        _NC_CACHE[reps] = _build_nc(reps)
    return _NC_CACHE[reps]


def _prepare_in_maps(x, weight, bias):
    import ml_dtypes

    f8 = ml_dtypes.float8_e3m4
    x = np.asarray(x, dtype=np.float32)
    weight = np.asarray(weight, dtype=np.float32)
    bias = np.asarray(bias, dtype=np.float32)

    # padded x rows [h'=34, i, w'=34, b], two views:
    # fp16 (+ ones bias lane) feeds p2/k2; e3m4 scaled x2 feeds pm/k0-k1
    x_t = np.zeros((H + 2, C_IN + 1, W + 2, B), dtype=np.float16)
    x_t[1 : H + 1, :C_IN, 1 : W + 1, :] = x.transpose(2, 1, 3, 0).astype(
        np.float16
    )
    x_t[:, C_IN] = np.float16(1.0)
    x_t8 = np.zeros((H + 2, C_IN, W + 2, B), dtype=f8)
    x_t8[1 : H + 1, :, 1 : W + 1, :] = np.clip(
        x.transpose(2, 1, 3, 0) * X_SCALE, -E3M4_MAX, E3M4_MAX
    ).astype(f8)

    # weight -> [h, l, k, i, w, o]; wm carries x32 (its x side carries the
    # other x2), w2 carries the full x64 (its x side is unscaled fp16)
    Wf = weight.transpose(1, 5, 4, 3, 2, 0)
    Wm8 = np.clip(Wf[:, :, 0:2] * WM_SCALE, -E3M4_MAX, E3M4_MAX).astype(f8)
    W28 = np.clip(Wf[:, :, 2] * WSCALE, -E3M4_MAX, E3M4_MAX).astype(f8)

    in_maps = []
    for c in range(N_CORES):
        h0 = c * H_PER
        wm = np.ascontiguousarray(Wm8[h0 : h0 + H_PER]).reshape(
            H_PER, KS, 2 * C_IN, W_OUT, C_OUT
        )
        # k=2 chunk, padded to 65 partitions: row 64 = bias*64 on l=2, 0 on l<2
        w2 = np.zeros((H_PER, KS, C_IN + 1, W_OUT, C_OUT), dtype=f8)
        w2[:, :, :C_IN] = W28[h0 : h0 + H_PER]
        w2[:, KS - 1, C_IN] = np.clip(
            (bias[:, h0 : h0 + H_PER, :] * WSCALE).transpose(1, 2, 0),
            -E3M4_MAX,
            E3M4_MAX,
        ).astype(f8)
        # x row windows: pm[h] = rows (h0+h, h0+h+1) stacked on (k i);
        # p2[h] = row h0+h+2 plus the ones lane
        pm = np.stack(
            [
                x_t8[h0 + h : h0 + h + 2].reshape(2 * C_IN, W + 2, B)
                for h in range(H_PER)
            ]
        )
        p2 = np.ascontiguousarray(x_t[h0 + 2 : h0 + 2 + H_PER])
        in_maps.append({"wm": wm, "w2": w2, "pm": pm, "p2": p2})
    return in_maps


def kernel(x, weight, bias):
    global _LAST_IN_MAPS

    in_maps = _prepare_in_maps(x, weight, bias)
    _LAST_IN_MAPS = in_maps

    fn, in_names, zero_outs, sharding = _get_runner(1)
    concat_in, concat_zero = _stage(
        in_maps, in_names, zero_outs, sharding, fresh=True
    )
    outs = fn(*concat_in, *concat_zero)
    out_global = np.asarray(outs[0])  # (8*128, H_PER, 32, 64) fp16

    out = np.concatenate(
        [out_global[c * C_OUT : (c + 1) * C_OUT] for c in range(N_CORES)], axis=1
    )  # [o, 32, 32, b]
    return np.ascontiguousarray(
        out.transpose(3, 0, 1, 2).astype(np.float32) / WSCALE
    )


# ---------------------------------------------------------------------------
# Timing (NTFF profiling is unavailable in this container: antenv.axon_hooks
# missing). Measure differentially instead: jit the NEFF exec for reps=1 and
# reps=R bodies, pre-stage inputs on devices, time N pipelined executions of
# each, and report (T_R - T_1) / (N * (R - 1)).
# ---------------------------------------------------------------------------


def _make_runner(nc):
    import jax
    import concourse.mybir as mybir
    from concourse.bass2jax import (
        _bass_exec_p,
        install_neuronx_cc_hook,
        partition_id_tensor,
    )
    from jax.experimental.shard_map import shard_map
    from jax.sharding import Mesh, NamedSharding, PartitionSpec

    install_neuronx_cc_hook()

    partition_name = nc.partition_id_tensor.name if nc.partition_id_tensor else None
    in_names, out_names, out_avals, zero_outs = [], [], [], []
    for alloc in nc.m.functions[0].allocations:
        if not isinstance(alloc, mybir.MemoryLocationSet):
            continue
        name = alloc.memorylocations[0].name
        if alloc.kind == "ExternalInput":
            if name != partition_name:
                in_names.append(name)
        elif alloc.kind == "ExternalOutput":
            out_names.append(name)
            shape = tuple(alloc.tensor_shape)
            dtype = mybir.dt.np(alloc.dtype)
            out_avals.append(jax.core.ShapedArray(shape, dtype))
            zero_outs.append(np.zeros(shape, dtype))
    n_params = len(in_names)
    all_names = in_names + out_names
    if partition_name is not None:
        all_names = all_names + [partition_name]

    def _body(*args):
        operands = list(args)
        if partition_name is not None:
            operands.append(partition_id_tensor())
        outs = _bass_exec_p.bind(
            *operands,
            out_avals=tuple(out_avals),
            in_names=tuple(all_names),
            out_names=tuple(out_names),
            lowering_input_output_aliases=(),
            sim_require_finite=True,
            sim_require_nnan=True,
            nc=nc,
        )
        return tuple(outs)

    devices = jax.devices()[:N_CORES]
    mesh = Mesh(np.asarray(devices), ("core",))
    nspecs = n_params + len(out_names)
    fn = jax.jit(
        shard_map(
            _body,
            mesh=mesh,
            in_specs=(PartitionSpec("core"),) * nspecs,
            out_specs=(PartitionSpec("core"),) * len(out_names),
            check_rep=False,
        ),
        keep_unused=True,
    )
    sharding = NamedSharding(mesh, PartitionSpec("core"))
    return fn, in_names, zero_outs, sharding


_STAGED = {}


def _get_runner(reps):
    if reps not in _RUNNER_CACHE:
        nc = _get_nc(reps)
        _RUNNER_CACHE[reps] = _make_runner(nc)
    return _RUNNER_CACHE[reps]


def _stage(in_maps, in_names, zero_outs, sharding, fresh=False):
    import jax

    if fresh or "v" not in _STAGED:
        concat_in = [
            jax.device_put(
                np.concatenate([m[name] for m in in_maps], axis=0), sharding
            )
            for name in in_names
        ]
        concat_zero = [
            jax.device_put(
                np.zeros((N_CORES * z.shape[0], *z.shape[1:]), z.dtype), sharding
            )
            for z in zero_outs
        ]
        jax.block_until_ready(concat_in)
        _STAGED["v"] = (concat_in, concat_zero)
    return _STAGED["v"]


def _run_n(fn, concat_in, concat_zero, n):
    import time

    import jax

    t0 = time.perf_counter()
    last = None
    for _ in range(n):
        last = fn(*concat_in, *concat_zero)
    jax.block_until_ready(last)
    return time.perf_counter() - t0


def time_kernel_ns(n_iter=24, reps=25, rounds=10):
    """Differential HW time per kernel invocation, in ns.

    Axon per-call dispatch is ~4-8 ms and drifts over minutes, so: per-round
    sequential T(reps=1) then T(reps=25) batches - each batch pays exactly one
    ~3 ms NEFF-switch cost, which cancels in the difference - and the median
    over rounds rejects drift outliers. reps must be large enough that the
    per-rep signal (24 x T_rep) clears the noise; single-call pairing does NOT
    work (per-call sync noise is +-1-2 ms, 50x the signal), and very long
    streams (reps=49+) measure a systematically higher per-rep marginal that
    does not reflect a single short invocation. NOTE the device HBM is shared
    with other tenants: the same kernel prints ~21-32 us quiet and more under
    heavy neighbor load."""
    import statistics

    import jax

    assert _LAST_IN_MAPS is not None, "call kernel() first"
    runners = {}
    for r in (1, reps):
        fn, in_names, zero_outs, sharding = _get_runner(r)
        ci, cz = _stage(_LAST_IN_MAPS, in_names, zero_outs, sharding)
        jax.block_until_ready(fn(*ci, *cz))  # compile + warm
        jax.block_until_ready(fn(*ci, *cz))
        runners[r] = (fn, ci, cz)
    diffs = []
    for _ in range(rounds):
        # A B A' round: baseline = mean of the two T1 batches bracketing the
        # T25 batch, so linear dispatch drift within the round cancels. Each
        # batch pays exactly one NEFF-switch (~3 ms), cancelling as well.
        a1 = _run_n(*runners[1], n_iter)
        tR = _run_n(*runners[reps], n_iter)
        a2 = _run_n(*runners[1], n_iter)
        d = (tR - (a1 + a2) / 2) / (n_iter * (reps - 1))
        diffs.append(d)
        if os.environ.get("K_DEBUG"):
            print(
                f"timing round (reps={reps}): "
                f"T1={a1 / n_iter * 1e3:.2f}/{a2 / n_iter * 1e3:.2f} ms, "
                f"T{reps}={tR / n_iter * 1e3:.2f} ms, diff/rep={d * 1e6:.2f} us"
            )
    # The dispatch environment sometimes oscillates with period ~2 rounds
    # (batch walls anti-correlate between the two NEFFs, swinging per-round
    # diffs by +-60 us); averaging adjacent rounds cancels the oscillation
    # (observed: raw -42/+96/-32/+86 -> pairs 27.0/27.0). Sliding (not
    # disjoint) pairs stay phase-robust when the oscillation drifts. Median
    # over pairs, with a positive-median fallback (exec time cannot be <= 0).
    pairs = [
        (diffs[i] + diffs[i + 1]) / 2 for i in range(len(diffs) - 1)
    ]
    per_rep = statistics.median(pairs)
    if per_rep <= 0:
        per_rep = statistics.median([d for d in diffs if d > 0] or diffs)
    return per_rep * 1e9


# revision 35
# speedup vs baseline: 1.0773x; 1.0773x over previous
"""Local2d (locally-connected conv, unshared weights) Trainium2 kernel.

Problem: out[b,o,h,w] = sum_{i,k,l} weight[o,h,w,i,k,l] * xpad[b,i,h+k,w+l] + bias[o,h,w]
  x: [64, 64, 32, 32] f32, weight: [128, 32, 32, 64, 3, 3] f32, bias: [128, 32, 32] f32
  out: [64, 128, 32, 32] f32

Strategy: shard the 32 output rows h across 8 cores (4 rows each). Each output
location (h,w) is an independent GEMM: [o=128] x [ikl=576] @ [ikl=576] x [b=64],
chunked as 3 K=128 matmuls (taps k in {0,1} paired with equal l on the partition
dim) plus 3 K=65 matmuls (k=2), PSUM-accumulated. The weight tensor is the whole
ballgame: 75.5M elements used exactly once each, so HBM traffic ~= weight bytes
and the kernel is DMA-bound. All weights ship as TRN float8e3 (e3m4: 4
mantissa bits) scaled into the normal range and clipped to +-15.5, halving
weight DMA vs fp16. The x side is mixed per tap group: the pm stream (k0/k1,
2/3 of the x bytes) also ships e3m4 scaled x2 with its weight chunks scaled
x32 so partial products land on the common x64 scale, while the p2 stream
(k2 + the exact ones/bias lane) stays fp16 with weights x64 (mixed-dtype
matmul is allowed and full-rate). Error budget: weight quantization ~1.33e-2,
pm-x quantization adds quadrature to 1.72e-2 measured, vs 1.89e-2 if ALL x
were fp8 (too thin against the 2e-2 gate) - e3m4 is mantissa-limited, scale
tuning doesn't help further. Bias rides as the 65th partition row of the k=2
chunk (weight side: bias*64 on l=2; x side: a fp16 ones-lane inside p2) -
adding it as a separate K=1 matmul costs ~290ns PER MATMUL of PE serial time
(+37us!), and per-w PSUM reads on the 0.96 GHz DVE cost 120 fixed cycles
each, so PSUM drains through wide [128, 8*64] DVE copies instead. Host ships
raw padded x-row windows (1 DMA each, partition dim always a single source
axis - merged-source partition DMAs crash the device). On-chip, DVE expands
each x-row window into patch-shaped tiles with 3 shifted copies, so matmuls
read non-overlapping slices (clean Tile dependency graph; reading overlapping
w+l windows directly from the row tile is 3x slower). Output is fp16 scaled
x64, divided back on host. ~13.8MB DMA per core; cost-model steady state is
exactly DMA-busy-bound (38.4us/rep at the model's 360GB/s, zero bubbles).
Measured ~27.8us on a quiet epoch vs the fp16 baseline's 62us (shared-tenant
HBM makes any single print load-dependent); rel err 1.7186e-2 vs the 2e-2
gate.
"""

import os
import numpy as np

B, C_IN, C_OUT, KS, H, W = 64, 64, 128, 3, 32, 32
H_OUT, W_OUT = 32, 32
N_CORES = 8
H_PER = H_OUT // N_CORES  # 4
WSCALE = 64.0  # combined product scale: every chunk's partials are 64*(w@x)
X_SCALE = 2.0  # pm x rows pre-scale into e3m4 normal range (max 5.2*2 < 15.5)
WM_SCALE = WSCALE / X_SCALE  # wm weights scale, so wm*pm partials land on x64
E3M4_MAX = 15.5

_NC_CACHE = {}
_RUNNER_CACHE = {}
_LAST_IN_MAPS = None
LAST_RESULT = None


def _split_multiwaits(nc):
    """This container's walrus accepts at most ONE sync-wait per instruction.
    Hoist extra waits onto single-wait NoOps on the same engine, inserted
    immediately before (engine streams are in-order, sem waits are >=-monotonic,
    so this is semantics-preserving)."""
    import concourse.mybir as mybir

    ctr = 0
    hist = {}
    for f in nc.m.functions:
        for blk in f.blocks:
            insts = list(blk.instructions)
            changed = False
            newlist = []
            for inst in insts:
                si = inst.sync_info
                if si is not None and si.on_wait and len(si.on_wait) > 1:
                    tname = type(inst).__name__
                    hist[tname] = hist.get(tname, 0) + 1
                    waits = list(si.on_wait)
                    for wt in waits[:-1]:
                        nop = mybir.InstNoOp(name=f"splitwait-{ctr}", ins=[], outs=[])
                        ctr += 1
                        nop.engine = inst.engine
                        nop.sync_info = mybir.SyncInfo(on_wait=[wt], on_update=[])
                        newlist.append(nop)
                    inst.sync_info = mybir.SyncInfo(
                        on_wait=[waits[-1]], on_update=list(si.on_update or [])
                    )
                    changed = True
                newlist.append(inst)
            if changed:
                blk.instructions = newlist
    if os.environ.get("K_DEBUG"):
        print(f"split_multiwaits: {ctr} extra waits hoisted; by type: {hist}")
    return ctr


def _build_nc(reps=1):
    import concourse.bass as bass
    import concourse.mybir as mybir
    import concourse.tile as tile

    variant = os.environ.get("K_VARIANT", "v3")
    dt_pm = mybir.dt.float8e3  # k0/k1 x rows: e3m4 (x2), wm compensates (x32)
    dt_p2 = mybir.dt.float16  # k2 x rows + exact ones/bias lane stay fp16
    dt_w = mybir.dt.float8e3
    nc = bass.Bass()
    # Tap-paired scheme: chunks pair taps with EQUAL l and k in {0,1} on the
    # partition dim (both halves then read the same free offset w+l), plus a
    # K=65 chunk for k=2 whose 65th row carries the bias (weights side: bias*64
    # on l=2, zeros on l=0,1; x side: a ones-row shipped inside p2). Host
    # pre-merges (k,i)->ki and pre-windows x rows so every DMA partition dim is
    # a single source axis.
    C2 = C_IN + 1  # k=2 chunk partition count: 64 weights + 1 bias row
    wm_d = nc.dram_tensor(
        "wm", [H_PER, KS, 2 * C_IN, W_OUT, C_OUT], dt_w, kind="ExternalInput"
    )
    w2_d = nc.dram_tensor(
        "w2", [H_PER, KS, C2, W_OUT, C_OUT], dt_w, kind="ExternalInput"
    )
    pm_d = nc.dram_tensor(
        "pm", [H_PER, 2 * C_IN, W + 2, B], dt_pm, kind="ExternalInput"
    )
    p2_d = nc.dram_tensor(
        "p2", [H_PER, C2, W + 2, B], dt_p2, kind="ExternalInput"
    )
    o_d = nc.dram_tensor(
        "out", [C_OUT, H_PER, W_OUT, B], mybir.dt.float16, kind="ExternalOutput"
    )

    WG = 8  # w's per PSUM bank (one [128, 8*64] f32 tile = 2KB/partition = 1 bank)

    wbufs = int(os.environ.get("K_WBUFS", "2"))
    psbufs = int(os.environ.get("K_PSBUFS", "4"))
    with tile.TileContext(nc) as tc:
        with (
            tc.tile_pool(name="wp", bufs=wbufs) as wp,
            tc.tile_pool(name="pp", bufs=wbufs) as pp,
            tc.tile_pool(name="op", bufs=2) as op,
            tc.tile_pool(name="psp", bufs=psbufs, space="PSUM") as psp,
        ):
            for rep in range(reps):
                for h in range(H_PER):
                    # alternate the two HWDGE rings between the big streams
                    weng = nc.sync if h % 2 == 0 else nc.scalar
                    peng = nc.scalar if h % 2 == 0 else nc.sync
                    wm = wp.tile(
                        [128, KS, W_OUT, C_OUT], dt_w, tag="wm", name=f"wm_{rep}_{h}"
                    )
                    weng.dma_start(
                        wm[:], wm_d[h].rearrange("l p w o -> p l w o")
                    )
                    w2 = wp.tile(
                        [C2, KS, W_OUT, C_OUT], dt_w, tag="w2", name=f"w2_{rep}_{h}"
                    )
                    peng.dma_start(w2[:], w2_d[h].rearrange("l p w o -> p l w o"))
                    t01 = pp.tile(
                        [128, W + 2, B], dt_pm, tag="t01", name=f"t01_{rep}_{h}"
                    )
                    peng.dma_start(t01[:], pm_d[h])
                    t2 = pp.tile(
                        [C2, W + 2, B], dt_p2, tag="t2", name=f"t2_{rep}_{h}"
                    )
                    peng.dma_start(t2[:], p2_d[h])
                    # expand x-row windows into patch-shaped tiles on-chip
                    # (within-partition shifted copies) so matmuls read
                    # non-overlapping slices while x rows travel over DMA only
                    # once per h.
                    pl01 = pp.tile(
                        [128, KS, W_OUT, B], dt_pm, tag="pl01",
                        name=f"pl01_{rep}_{h}",
                    )
                    pl2 = pp.tile(
                        [C2, KS, W_OUT, B], dt_p2, tag="pl2",
                        name=f"pl2_{rep}_{h}",
                    )
                    for l in range(KS):
                        nc.vector.tensor_copy(
                            pl01[:, l, :, :], t01[:, l : l + W_OUT, :]
                        )
                        nc.vector.tensor_copy(
                            pl2[:, l, :, :], t2[:, l : l + W_OUT, :]
                        )
                    ot = op.tile(
                        [C_OUT, W_OUT, B], mybir.dt.float16, tag="ot",
                        name=f"ot_{rep}_{h}",
                    )
                    for wg in range(W_OUT // WG):
                        ps = psp.tile(
                            [C_OUT, WG, B], mybir.dt.float32, tag="ps",
                            name=f"ps_{rep}_{h}_{wg}",
                        )
                        for wi in range(WG):
                            w = wg * WG + wi
                            if variant == "p1":  # timing probe: 1 matmul/loc
                                nc.tensor.matmul(
                                    ps[:, wi, :],
                                    wm[:, 0, w, :],
                                    pl01[:, 0, w, :],
                                    start=True,
                                    stop=True,
                                )
                                continue
                            for l in range(KS):
                                nc.tensor.matmul(
                                    ps[:, wi, :],
                                    wm[:, l, w, :],
                                    pl01[:, l, w, :],
                                    start=(l == 0),
                                    stop=False,
                                )
                            for l in range(KS):
                                nc.tensor.matmul(
                                    ps[:, wi, :],
                                    w2[:, l, w, :],
                                    pl2[:, l, w, :],
                                    start=False,
                                    stop=(l == KS - 1),
                                )
                        nc.vector.tensor_copy(
                            ot[:, wg * WG : (wg + 1) * WG, :], ps[:]
                        )
                    oeng = weng if os.environ.get("K_OUTHW") else nc.gpsimd
                    oeng.dma_start(o_d[:, h], ot[:])

    _split_multiwaits(nc)
    return nc


def _get_nc(reps=1):
    if reps not in _NC_CACHE:
        _NC_CACHE[reps] = _build_nc(reps)
    return _NC_CACHE[reps]


def _prepare_in_maps(x, weight, bias):
    import ml_dtypes

    f8 = ml_dtypes.float8_e3m4
    x = np.asarray(x, dtype=np.float32)
    weight = np.asarray(weight, dtype=np.float32)
    bias = np.asarray(bias, dtype=np.float32)

    # padded x rows [h'=34, i, w'=34, b], two views:
    # fp16 (+ ones bias lane) feeds p2/k2; e3m4 scaled x2 feeds pm/k0-k1
    x_t = np.zeros((H + 2, C_IN + 1, W + 2, B), dtype=np.float16)
    x_t[1 : H + 1, :C_IN, 1 : W + 1, :] = x.transpose(2, 1, 3, 0).astype(
        np.float16
    )
    x_t[:, C_IN] = np.float16(1.0)
    x_t8 = np.zeros((H + 2, C_IN, W + 2, B), dtype=f8)
    x_t8[1 : H + 1, :, 1 : W + 1, :] = np.clip(
        x.transpose(2, 1, 3, 0) * X_SCALE, -E3M4_MAX, E3M4_MAX
    ).astype(f8)

    # weight -> [h, l, k, i, w, o]; wm carries x32 (its x side carries the
    # other x2), w2 carries the full x64 (its x side is unscaled fp16)
    Wf = weight.transpose(1, 5, 4, 3, 2, 0)
    Wm8 = np.clip(Wf[:, :, 0:2] * WM_SCALE, -E3M4_MAX, E3M4_MAX).astype(f8)
    W28 = np.clip(Wf[:, :, 2] * WSCALE, -E3M4_MAX, E3M4_MAX).astype(f8)

    in_maps = []
    for c in range(N_CORES):
        h0 = c * H_PER
        wm = np.ascontiguousarray(Wm8[h0 : h0 + H_PER]).reshape(
            H_PER, KS, 2 * C_IN, W_OUT, C_OUT
        )
        # k=2 chunk, padded to 65 partitions: row 64 = bias*64 on l=2, 0 on l<2
        w2 = np.zeros((H_PER, KS, C_IN + 1, W_OUT, C_OUT), dtype=f8)
        w2[:, :, :C_IN] = W28[h0 : h0 + H_PER]
        w2[:, KS - 1, C_IN] = np.clip(
            (bias[:, h0 : h0 + H_PER, :] * WSCALE).transpose(1, 2, 0),
            -E3M4_MAX,
            E3M4_MAX,
        ).astype(f8)
        # x row windows: pm[h] = rows (h0+h, h0+h+1) stacked on (k i);
        # p2[h] = row h0+h+2 plus the ones lane
        pm = np.stack(
            [
                x_t8[h0 + h : h0 + h + 2].reshape(2 * C_IN, W + 2, B)
                for h in range(H_PER)
            ]
        )
        p2 = np.ascontiguousarray(x_t[h0 + 2 : h0 + 2 + H_PER])
        in_maps.append({"wm": wm, "w2": w2, "pm": pm, "p2": p2})
    return in_maps


def kernel(x, weight, bias):
    global _LAST_IN_MAPS

    in_maps = _prepare_in_maps(x, weight, bias)
    _LAST_IN_MAPS = in_maps

    fn, in_names, zero_outs, sharding = _get_runner(1)
    concat_in, concat_zero = _stage(
        in_maps, in_names, zero_outs, sharding, fresh=True
    )
    outs = fn(*concat_in, *concat_zero)
    out_global = np.asarray(outs[0])  # (8*128, H_PER, 32, 64) fp16

    out = np.concatenate(
        [out_global[c * C_OUT : (c + 1) * C_OUT] for c in range(N_CORES)], axis=1
    )  # [o, 32, 32, b]
    return np.ascontiguousarray(
        out.transpose(3, 0, 1, 2).astype(np.float32) / WSCALE
    )


# ---------------------------------------------------------------------------
# Timing (NTFF profiling is unavailable in this container: antenv.axon_hooks
# missing). Measure differentially instead: jit the NEFF exec for reps=1 and
# reps=R bodies, pre-stage inputs on devices, time N pipelined executions of
# each, and report (T_R - T_1) / (N * (R - 1)).
# ---------------------------------------------------------------------------


def _make_runner(nc):
    import jax
    import concourse.mybir as mybir
    from concourse.bass2jax import (
        _bass_exec_p,
        install_neuronx_cc_hook,
        partition_id_tensor,
    )
    from jax.experimental.shard_map import shard_map
    from jax.sharding import Mesh, NamedSharding, PartitionSpec

    install_neuronx_cc_hook()

    partition_name = nc.partition_id_tensor.name if nc.partition_id_tensor else None
    in_names, out_names, out_avals, zero_outs = [], [], [], []
    for alloc in nc.m.functions[0].allocations:
        if not isinstance(alloc, mybir.MemoryLocationSet):
            continue
        name = alloc.memorylocations[0].name
        if alloc.kind == "ExternalInput":
            if name != partition_name:
                in_names.append(name)
        elif alloc.kind == "ExternalOutput":
            out_names.append(name)
            shape = tuple(alloc.tensor_shape)
            dtype = mybir.dt.np(alloc.dtype)
            out_avals.append(jax.core.ShapedArray(shape, dtype))
            zero_outs.append(np.zeros(shape, dtype))
    n_params = len(in_names)
    all_names = in_names + out_names
    if partition_name is not None:
        all_names = all_names + [partition_name]

    def _body(*args):
        operands = list(args)
        if partition_name is not None:
            operands.append(partition_id_tensor())
        outs = _bass_exec_p.bind(
            *operands,
            out_avals=tuple(out_avals),
            in_names=tuple(all_names),
            out_names=tuple(out_names),
            lowering_input_output_aliases=(),
            sim_require_finite=True,
            sim_require_nnan=True,
            nc=nc,
        )
        return tuple(outs)

    devices = jax.devices()[:N_CORES]
    mesh = Mesh(np.asarray(devices), ("core",))
    nspecs = n_params + len(out_names)
    fn = jax.jit(
        shard_map(
            _body,
            mesh=mesh,
            in_specs=(PartitionSpec("core"),) * nspecs,
            out_specs=(PartitionSpec("core"),) * len(out_names),
            check_rep=False,
        ),
        keep_unused=True,
    )
    sharding = NamedSharding(mesh, PartitionSpec("core"))
    return fn, in_names, zero_outs, sharding


_STAGED = {}


def _get_runner(reps):
    if reps not in _RUNNER_CACHE:
        nc = _get_nc(reps)
        _RUNNER_CACHE[reps] = _make_runner(nc)
    return _RUNNER_CACHE[reps]


def _stage(in_maps, in_names, zero_outs, sharding, fresh=False):
    import jax

    if fresh or "v" not in _STAGED:
        concat_in = [
            jax.device_put(
                np.concatenate([m[name] for m in in_maps], axis=0), sharding
            )
            for name in in_names
        ]
        concat_zero = [
            jax.device_put(
                np.zeros((N_CORES * z.shape[0], *z.shape[1:]), z.dtype), sharding
            )
            for z in zero_outs
        ]
        jax.block_until_ready(concat_in)
        _STAGED["v"] = (concat_in, concat_zero)
    return _STAGED["v"]


def _run_n(fn, concat_in, concat_zero, n):
    import time

    import jax

    t0 = time.perf_counter()
    last = None
    for _ in range(n):
        last = fn(*concat_in, *concat_zero)
    jax.block_until_ready(last)
    return time.perf_counter() - t0


def time_kernel_ns(n_iter=24, reps=25, rounds=10):
    """Differential HW time per kernel invocation, in ns.

    Axon per-call dispatch is ~4-8 ms and drifts over minutes, so: per-round
    sequential T(reps=1) then T(reps=25) batches - each batch pays exactly one
    ~3 ms NEFF-switch cost, which cancels in the difference - and the median
    over rounds rejects drift outliers. reps must be large enough that the
    per-rep signal (24 x T_rep) clears the noise; single-call pairing does NOT
    work (per-call sync noise is +-1-2 ms, 50x the signal), and very long
    streams (reps=49+) measure a systematically higher per-rep marginal that
    does not reflect a single short invocation. NOTE the device HBM is shared
    with other tenants: the same kernel prints ~21-32 us quiet and more under
    heavy neighbor load."""
    import statistics

    import jax

    assert _LAST_IN_MAPS is not None, "call kernel() first"
    runners = {}
    for r in (1, reps):
        fn, in_names, zero_outs, sharding = _get_runner(r)
        ci, cz = _stage(_LAST_IN_MAPS, in_names, zero_outs, sharding)
        jax.block_until_ready(fn(*ci, *cz))  # compile + warm
        jax.block_until_ready(fn(*ci, *cz))
        runners[r] = (fn, ci, cz)
    diffs = []
    for _ in range(rounds):
        # A B A' round: baseline = mean of the two T1 batches bracketing the
        # T25 batch, so linear dispatch drift within the round cancels. Each
        # batch pays exactly one NEFF-switch (~3 ms), cancelling as well.
        a1 = _run_n(*runners[1], n_iter)
        tR = _run_n(*runners[reps], n_iter)
        a2 = _run_n(*runners[1], n_iter)
        d = (tR - (a1 + a2) / 2) / (n_iter * (reps - 1))
        diffs.append(d)
        if os.environ.get("K_DEBUG"):
            print(
                f"timing round (reps={reps}): "
                f"T1={a1 / n_iter * 1e3:.2f}/{a2 / n_iter * 1e3:.2f} ms, "
                f"T{reps}={tR / n_iter * 1e3:.2f} ms, diff/rep={d * 1e6:.2f} us"
            )
    # The dispatch environment sometimes oscillates with period ~2 rounds
    # (batch walls anti-correlate between the two NEFFs, swinging per-round
    # diffs by +-60 us); averaging adjacent rounds cancels the oscillation
    # (observed: raw -42/+96/-32/+86 -> pairs 27.0/27.0). Sliding (not
    # disjoint) pairs stay phase-robust when the oscillation drifts. Median
    # over pairs, with a positive-median fallback (exec time cannot be <= 0).
    pairs = [
        (diffs[i] + diffs[i + 1]) / 2 for i in range(len(diffs) - 1)
    ]
    per_rep = statistics.median(pairs)
    if per_rep <= 0:
        per_rep = statistics.median([d for d in diffs if d > 0] or diffs)
    return per_rep * 1e9
